# revision 32
# baseline (speedup 1.0000x reference)
"""Bass/Trainium2 kernel for nn_Encoder_Flows (4-layer SAGE encoder with
buggy prefix-mean aggregation), SPMD over 8 NeuronCores.

Math (per reference):
  x = flow_matrix.reshape(B*K, P)   # [32768, 1024]
  4x: out = agg @ w_l.T + b_l + x @ w_r.T ; out /= ||out||_row
  where agg[j] = mean_{i<j} x[i] for j < K=1024, else 0.
  final relu.

Strategy:
  - Shard the 32768 rows into 8 contiguous shards of 4096 (core c gets rows
    [4096c, 4096(c+1))). Rows >= 1024 are row-independent (agg = 0).
  - Feature-major on chip: activations live as A[d, cols]; matmuls are then
    always out[dout_tile, cols] = W_T_slice.T @ A with no transposes.
  - The prefix-mean for rows < 1024 (core 0 only) is a matmul against a
    lower-triangular coefficient matrix M (L[j,i] = 1/j, i<j):
      C = G contracted against M.T, with G = x_k @ w_l.T computed row-major
      (its lhsT is exactly the feature-major activation tile).
    Cores 1-7 get M = 0 (same SPMD code, zero contribution).
  - fp16 matmul operands (full PE rate), fp32 PSUM accumulate, fp32 output.
  - All 8 column-chunks march through the layers together, stage by stage
    (the Tile scheduler follows emission order per engine, so emission
    interleaving IS the software pipeline).
  - Row norm (per free-dim column): bias+copy to fp16 (frees PSUM fast),
    square on DVE, column-sum via ones-vector matmul on PE, sqrt on ACT,
    reciprocal on DVE on a [128, CH/128] refold (a [1, CH] strip would be
    single-lane), partition-broadcast on GpSimd, fused multiply on DVE.
"""

import sys

if "/opt/trn_rl_repo" not in sys.path:
    sys.path.insert(0, "/opt/trn_rl_repo")

import numpy as np

B, K, P = 32, 1024, 1024
N_CORES = 8
RPC = (B * K) // N_CORES  # 4096 columns (rows of x) per core
CH = 512                  # chunk of columns processed at once
NCH = RPC // CH           # 8 chunks; chunks 0,1 hold the coupled rows 0..1023
DIMS = [(1024, 128), (128, 256), (256, 128), (128, 256)]
DOUT = DIMS[-1][1]

# nonzero [128i, 512j] blocks of M.T (MT[i,j] = 1/j if i<j else 0)
MT_BLOCKS = [(it, 0) for it in range(4)] + [(it, 1) for it in range(8)]


def _mt_block_id(it, jc):
    return it if jc == 0 else 4 + it


_CACHE = {}


def _build_program():
    import concourse.bass as bass  # noqa: F401
    import concourse.tile as tile
    from concourse import bacc, mybir

    f16 = mybir.dt.float16
    f32 = mybir.dt.float32
    AF = mybir.ActivationFunctionType
    OP = mybir.AluOpType

    nc = bacc.Bacc("TRN2", target_bir_lowering=False, debug=False)

    xt = nc.dram_tensor("xt", [P, RPC], f16, kind="ExternalInput").ap()
    mt = nc.dram_tensor("mt", [128, len(MT_BLOCKS) * CH], f16,
                        kind="ExternalInput").ap()
    wr_d, wl_d, b_d = [], [], []
    for li, (din, dout) in enumerate(DIMS):
        kt, pt = din // 128, dout // 128
        # host pre-packs weights partition-major so each load is one
        # contiguous row per partition
        wr_d.append(nc.dram_tensor(f"wr{li}", [128, kt * dout], f16,
                                   kind="ExternalInput").ap())
        wl_d.append(nc.dram_tensor(f"wl{li}", [128, kt * dout], f16,
                                   kind="ExternalInput").ap())
        b_d.append(nc.dram_tensor(f"b{li}", [128, pt], f32,
                                  kind="ExternalInput").ap())
    out_d = nc.dram_tensor("out", [DOUT, RPC], f32, kind="ExternalOutput").ap()
    out_r = out_d.rearrange("(pt p) c -> p pt c", p=128)

    with tile.TileContext(nc) as tc:
        with (
            tc.tile_pool(name="consts", bufs=1) as consts,
            tc.tile_pool(name="xk", bufs=1) as xkp,
            tc.tile_pool(name="xs", bufs=6) as xsp,
            tc.tile_pool(name="pa", bufs=1) as pap,
            tc.tile_pool(name="ab", bufs=13) as abp,
            tc.tile_pool(name="raw", bufs=8) as rawp,
            tc.tile_pool(name="gsb", bufs=8) as gsbp,
            tc.tile_pool(name="sq", bufs=8) as sqp,
            tc.tile_pool(name="sbc", bufs=8) as sbcp,
            tc.tile_pool(name="ost", bufs=3) as ostp,
            tc.tile_pool(name="mainp", bufs=4, space="PSUM") as mainp,
            tc.tile_pool(name="ssp", bufs=2, space="PSUM") as sspp,
            tc.tile_pool(name="gp", bufs=2, space="PSUM") as gpp,
        ):
            # ---- load order matters: the first PE work is layer-1 G
            # (needs wl0 + xk) then mains (wr0, then xs chunks) ----
            wr_sb = [None] * 4
            wl_sb = [None] * 4
            b_sb = [None] * 4

            def load_w(lst, dram, li, kt, dout, nm):
                w = consts.tile([128, kt, dout], f16, tag=f"{nm}{li}",
                                name=f"{nm}{li}")
                nc.sync.dma_start(
                    out=w, in_=dram[li].rearrange("p (k d) -> p k d", k=kt))
                lst[li] = w

            def load_b(li, dout):
                bt = consts.tile([128, dout // 128], f32, tag=f"b{li}",
                                 name=f"b{li}")
                nc.sync.dma_start(out=bt, in_=b_d[li])
                b_sb[li] = bt

            xs_sb = {}

            def load_xs(ch):
                x1 = xsp.tile([128, P // 128, CH], f16, tag="xs",
                              name=f"xs{ch}")
                nc.sync.dma_start(
                    out=x1,
                    in_=xt.rearrange("(k p) c -> p k c",
                                     p=128)[:, :, ch * CH:(ch + 1) * CH])
                xs_sb[ch] = x1

            # order matched to first consumers: layer-1 plain mains
            # (wr0 + xs2/3), then G (wl0 + xk), then C' (mt), the rest
            # while layer 1 runs
            load_w(wr_sb, wr_d, 0, 8, 128, "wr")
            load_b(0, 128)
            load_xs(2)
            load_xs(3)
            load_w(wl_sb, wl_d, 0, 8, 128, "wl")
            xk_sb = xkp.tile([128, P // 128, K], f16, tag="xk")
            nc.sync.dma_start(
                out=xk_sb,
                in_=xt.rearrange("(k p) c -> p k c", p=128)[:, :, 0:K])
            load_xs(4)
            mt_sb = consts.tile([128, len(MT_BLOCKS), CH], f16, tag="mt")
            nc.sync.dma_start(
                out=mt_sb,
                in_=mt.rearrange("p (b c) -> p b c", b=len(MT_BLOCKS)))
            for li, (din, dout) in enumerate(DIMS[1:], start=1):
                kt = din // 128
                load_w(wr_sb, wr_d, li, kt, dout, "wr")
                load_w(wl_sb, wl_d, li, kt, dout, "wl")
                load_b(li, dout)
            for ch in range(5, NCH):
                load_xs(ch)
            # all-ones stationary: the sumsq matmul then sums over features
            # AND broadcasts the result to every partition in one op
            ones128 = consts.tile([128, 128], f16, tag="ones128")
            nc.vector.memset(ones128, 1.0)
            warm_rhs = consts.tile([128, CH], f16, tag="warm_rhs")
            nc.vector.memset(warm_rhs, 0.0)

            # PE warmup: the HAM clock gate needs ~3.4us of sustained
            # activity to lift the PE to 2.4 GHz. Run throwaway matmuls on
            # memset data while the initial DMAs are in flight so the real
            # matmuls start warm.
            for wi in range(12):
                wp = sspp.tile([128, CH], f32, tag="ss", name=f"warm{wi}")
                nc.tensor.matmul(wp, lhsT=ones128, rhs=warm_rhs,
                                 start=True, stop=True)

            # persistent coupled activations per layer (columns 0..1023)
            pa_sb = []
            for li, (din, dout) in enumerate(DIMS[:-1]):
                pa_sb.append(pap.tile([128, dout // 128, K], f16,
                                      tag=f"pa{li}", name=f"pa{li}"))

            # per-chunk current activation APs: [128, kt, CH] views
            ain = {}
            for ch in range(NCH):
                if ch < 2:
                    ain[ch] = xk_sb[:, :, ch * CH:(ch + 1) * CH]
                else:
                    ain[ch] = xs_sb[ch]

            for li, (din, dout) in enumerate(DIMS):
                ktn = din // 128
                ptn = dout // 128
                is_last = li == 3
                gain = xk_sb if li == 0 else pa_sb[li - 1]

                # --- coupled G: G[i, f] = x_k @ w_l.T, row(i)-major ---
                g_sb = []

                def emit_g():
                    for it in range(8):
                        gp = gpp.tile([128, dout], f32, tag="gp", name="gp")
                        for kt in range(ktn):
                            nc.tensor.matmul(
                                gp, lhsT=gain[:, kt, it * 128:(it + 1) * 128],
                                rhs=wl_sb[li][:, kt, :],
                                start=(kt == 0), stop=(kt == ktn - 1))
                        g = gsbp.tile([128, dout], f16, tag="g", name="g")
                        nc.scalar.copy(g, gp)
                        g_sb.append(g)

                # at layer 1 the plain chunks' inputs land first; run their
                # mains before G so the PE starts as early as possible
                order = ([2, 3, 4, 5, 6, 7, 0, 1] if li == 0
                         else list(range(NCH)))

                # --- S0: mains (+ C' for coupled chunks) ---
                mains = {}
                for ch in order:
                    if ch < 2 and not g_sb:
                        emit_g()
                    for pt in range(ptn):
                        mp = mainp.tile([128, CH], f32, tag="mp",
                                        name=f"mp{ch}_{pt}")
                        for kt in range(ktn):
                            nc.tensor.matmul(
                                mp,
                                lhsT=wr_sb[li][:, kt, pt * 128:(pt + 1) * 128],
                                rhs=ain[ch][:, kt, :],
                                start=(kt == 0),
                                stop=(kt == ktn - 1 and ch >= 2))
                        if ch < 2:
                            its = [it for (it, j) in MT_BLOCKS if j == ch]
                            for ii, it in enumerate(its):
                                nc.tensor.matmul(
                                    mp,
                                    lhsT=g_sb[it][:, pt * 128:(pt + 1) * 128],
                                    rhs=mt_sb[:, _mt_block_id(it, ch), :],
                                    start=False, stop=(ii == len(its) - 1))
                        mains[(ch, pt)] = mp

                    # S1 immediately per chunk: raw16 = main + b (frees PSUM).
                    # Alternate ACT/DVE to balance engine load.
                    raw = rawp.tile([128, ptn, CH], f16, tag="raw",
                                    name=f"raw{ch}")
                    for pt in range(ptn):
                        if pt % 2 == 0:
                            nc.scalar.activation(
                                out=raw[:, pt, :], in_=mains[(ch, pt)],
                                func=AF.Identity,
                                bias=b_sb[li][:, pt:pt + 1], scale=1.0)
                        else:
                            nc.vector.tensor_scalar_add(
                                out=raw[:, pt, :], in0=mains[(ch, pt)],
                                scalar1=b_sb[li][:, pt:pt + 1])
                    mains[ch] = raw

                # --- S2: sq = (raw+b)^2 on DVE; S3: ss += ones.T @ sq ---
                # The sumsq matmuls trail the sq ops by two chunks so the
                # PE keeps streaming while early chunks' norm chains drain;
                # by the time the next layer's G matmuls need chunk 0/1's
                # outputs they are already done.
                sss = {}
                sqs = {}

                def emit_ss(ch):
                    ss = sspp.tile([128, CH], f32, tag="ss", name=f"ss{ch}")
                    for pt in range(ptn):
                        nc.tensor.matmul(ss, lhsT=ones128,
                                         rhs=sqs[ch][:, pt, :],
                                         start=(pt == 0), stop=(pt == ptn - 1))
                    sss[ch] = ss

                for idx, ch in enumerate(order):
                    raw = mains[ch]
                    sq = sqp.tile([128, ptn, CH], f16, tag="sq",
                                  name=f"sq{ch}")
                    for pt in range(ptn):
                        nc.vector.tensor_mul(
                            out=sq[:, pt, :], in0=raw[:, pt, :],
                            in1=raw[:, pt, :])
                    sqs[ch] = sq
                    if idx >= 2:
                        emit_ss(order[idx - 2])
                emit_ss(order[-2])
                emit_ss(order[-1])

                # --- S4: rsqrt of the broadcast sumsq, one wide ACT op ---
                sbs = {}
                for ch in order:
                    sb = sbcp.tile([128, CH], f16, tag="sbc", name=f"sb{ch}")
                    nc.scalar.activation(out=sb, in_=sss[ch],
                                         func=AF.Abs_reciprocal_sqrt)
                    sbs[ch] = sb

                # --- S7: apply scale ---
                for ch in order:
                    raw = mains[ch]
                    sb = sbs[ch]
                    if not is_last:
                        if ch < 2:
                            aout = pa_sb[li]
                            asl = (slice(None), slice(None),
                                   slice(ch * CH, (ch + 1) * CH))
                        else:
                            anext = abp.tile([128, ptn, CH], f16, tag="ab",
                                             name=f"ab{ch}")
                            aout = anext
                            asl = (slice(None), slice(None), slice(0, CH))
                            ain[ch] = anext
                        for pt in range(ptn):
                            dst = aout[asl[0], pt, asl[2]]
                            nc.vector.tensor_mul(
                                out=dst, in0=raw[:, pt, :], in1=sb)
                        if ch < 2:
                            ain[ch] = pa_sb[li][:, :,
                                               ch * CH:(ch + 1) * CH]
                    else:
                        ost = ostp.tile([128, ptn, CH], f32, tag="ost",
                                        name=f"ost{ch}")
                        for pt in range(ptn):
                            # relu((raw+b)*s) = max(raw+b,0)*s since s>0.
                            # These drain at the end of the kernel on DVE;
                            # hand a few to the otherwise-idle GpSimd
                            # (2 plain tensor_tensor ops there).
                            if pt == 1 and ch in (1, 3, 5):
                                gt = sqp.tile([128, CH], f16, tag="gt",
                                              name=f"gt{ch}")
                                nc.gpsimd.tensor_relu(gt, raw[:, pt, :])
                                nc.gpsimd.tensor_mul(
                                    out=ost[:, pt, :], in0=gt, in1=sb)
                            else:
                                nc.vector.scalar_tensor_tensor(
                                    out=ost[:, pt, :], in0=raw[:, pt, :],
                                    scalar=0.0, in1=sb,
                                    op0=OP.max, op1=OP.mult)
                        nc.sync.dma_start(
                            out=out_r[:, :, ch * CH:(ch + 1) * CH], in_=ost)

    nc.compile()
    return nc


def _prep_inputs(flow_matrix, ws):
    """ws: list of (w_l, b_l, w_r) fp32. Returns list of 8 in_maps."""
    x = np.ascontiguousarray(flow_matrix.reshape(B * K, P))
    xt_full = np.ascontiguousarray(x.T.astype(np.float16))  # [P, 32768]

    # M.T packed nonzero blocks, fp16
    inv = np.zeros(K, np.float32)
    inv[1:] = 1.0 / np.arange(1, K, dtype=np.float32)
    mt_packed = np.zeros((128, len(MT_BLOCKS) * CH), np.float16)
    for bid, (it, jc) in enumerate(MT_BLOCKS):
        i0, j0 = it * 128, jc * CH
        blk = np.zeros((128, CH), np.float32)
        for pp in range(128):
            i = i0 + pp
            jj = np.arange(j0, j0 + CH)
            blk[pp] = np.where(jj > i, inv[jj], 0.0)
        mt_packed[:, bid * CH:(bid + 1) * CH] = blk.astype(np.float16)
    mt_zero = np.zeros_like(mt_packed)

    def pack_w(wt):  # [din, dout] -> [128, kt*dout] partition-major
        din, dout = wt.shape
        kt = din // 128
        return np.ascontiguousarray(
            wt.reshape(kt, 128, dout).transpose(1, 0, 2).reshape(128, -1)
            .astype(np.float16))

    base = {}
    for li, (w_l, b_l, w_r) in enumerate(ws):
        base[f"wr{li}"] = pack_w(w_r.T)
        base[f"wl{li}"] = pack_w(w_l.T)
        base[f"b{li}"] = np.ascontiguousarray(
            b_l.reshape(-1, 128).T.astype(np.float32))

    in_maps = []
    for c in range(N_CORES):
        m = dict(base)
        m["xt"] = np.ascontiguousarray(xt_full[:, c * RPC:(c + 1) * RPC])
        m["mt"] = mt_packed if c == 0 else mt_zero
        in_maps.append(m)
    return in_maps


def kernel(flow_matrix, w_l1, b_l1, w_r1, w_l2, b_l2, w_r2,
           w_l3, b_l3, w_r3, w_l4, b_l4, w_r4, _trace=False, _tmpdir=None):
    from concourse import bass_utils

    flow_matrix = np.asarray(flow_matrix, dtype=np.float32)
    ws = [(np.asarray(w_l1, np.float32), np.asarray(b_l1, np.float32),
           np.asarray(w_r1, np.float32)),
          (np.asarray(w_l2, np.float32), np.asarray(b_l2, np.float32),
           np.asarray(w_r2, np.float32)),
          (np.asarray(w_l3, np.float32), np.asarray(b_l3, np.float32),
           np.asarray(w_r3, np.float32)),
          (np.asarray(w_l4, np.float32), np.asarray(b_l4, np.float32),
           np.asarray(w_r4, np.float32))]

    if "nc" not in _CACHE:
        _CACHE["nc"] = _build_program()
    nc = _CACHE["nc"]

    in_maps = _prep_inputs(flow_matrix, ws)
    res = bass_utils.run_bass_kernel_spmd(
        nc, in_maps, core_ids=list(range(N_CORES)), trace=_trace,
        tmpdir=_tmpdir)

    y = np.empty((B * K, DOUT), np.float32)
    for c in range(N_CORES):
        out_c = res.results[c]["out"]  # [DOUT, RPC] fp32
        y[c * RPC:(c + 1) * RPC, :] = out_c.T
    _CACHE["last_exec_time_ns"] = res.exec_time_ns
    return (y.reshape(B, K, DOUT), 1)


if __name__ == "__main__":
    rng = np.random.default_rng(0)
    fm = rng.standard_normal((B, K, P)).astype(np.float32)
    args = []
    for (din, dout) in DIMS:
        s = 1.0 / np.sqrt(din)
        args += [rng.uniform(-s, s, (dout, din)).astype(np.float32),
                 rng.uniform(-s, s, dout).astype(np.float32),
                 rng.uniform(-s, s, (dout, din)).astype(np.float32)]
    y, _ = kernel(fm, *args)
    print("ok", y.shape, y.dtype)


# revision 37
# speedup vs baseline: 1.2448x; 1.2448x over previous
"""Bass/Trainium2 kernel for nn_Encoder_Flows (4-layer SAGE encoder with
buggy prefix-mean aggregation), SPMD over 8 NeuronCores.

Math (per reference):
  x = flow_matrix.reshape(B*K, P)   # [32768, 1024]
  4x: out = agg @ w_l.T + b_l + x @ w_r.T ; out /= ||out||_row
  where agg[j] = mean_{i<j} x[i] for j < K=1024, else 0.
  final relu.

Strategy:
  - Shard the 32768 rows into 8 contiguous shards of 4096 (core c gets rows
    [4096c, 4096(c+1))). Rows >= 1024 are row-independent (agg = 0).
  - Feature-major on chip: activations live as A[d, cols]; matmuls are then
    always out[dout_tile, cols] = W_T_slice.T @ A with no transposes.
  - The prefix-mean for rows < 1024 (core 0 only) is a matmul against a
    lower-triangular coefficient matrix M (L[j,i] = 1/j, i<j):
      C = G contracted against M.T, with G = x_k @ w_l.T computed row-major
      (its lhsT is exactly the feature-major activation tile).
    Cores 1-7 get M = 0 (same SPMD code, zero contribution).
  - fp16 matmul operands (full PE rate), fp32 PSUM accumulate, fp32 output.
  - All 8 column-chunks march through the layers together, stage by stage
    (the Tile scheduler follows emission order per engine, so emission
    interleaving IS the software pipeline).
  - Row norm (per free-dim column): bias+copy to fp16 (frees PSUM fast),
    square on DVE, column-sum via ones-vector matmul on PE, sqrt on ACT,
    reciprocal on DVE on a [128, CH/128] refold (a [1, CH] strip would be
    single-lane), partition-broadcast on GpSimd, fused multiply on DVE.
"""

import sys

if "/opt/trn_rl_repo" not in sys.path:
    sys.path.insert(0, "/opt/trn_rl_repo")

import numpy as np

B, K, P = 32, 1024, 1024
N_CORES = 8
RPC = (B * K) // N_CORES  # 4096 columns (rows of x) per core
CH = 512                  # chunk of columns processed at once
NCH = RPC // CH           # 8 chunks; chunks 0,1 hold the coupled rows 0..1023
DIMS = [(1024, 128), (128, 256), (256, 128), (128, 256)]
DOUT = DIMS[-1][1]

# nonzero [128i, 512j] blocks of M.T (MT[i,j] = 1/j if i<j else 0)
MT_BLOCKS = [(it, 0) for it in range(4)] + [(it, 1) for it in range(8)]


def _mt_block_id(it, jc):
    return it if jc == 0 else 4 + it


_CACHE = {}


def _build_program():
    import concourse.bass as bass  # noqa: F401
    import concourse.tile as tile
    from concourse import bacc, mybir

    f16 = mybir.dt.float16
    f32 = mybir.dt.float32
    AF = mybir.ActivationFunctionType
    OP = mybir.AluOpType

    nc = bacc.Bacc("TRN2", target_bir_lowering=False, debug=False)

    xt = nc.dram_tensor("xt", [P, RPC], f16, kind="ExternalInput").ap()
    mt = nc.dram_tensor("mt", [128, len(MT_BLOCKS) * CH], f16,
                        kind="ExternalInput").ap()
    wr_d, wl_d, b_d = [], [], []
    for li, (din, dout) in enumerate(DIMS):
        kt, pt = din // 128, dout // 128
        # host pre-packs weights partition-major so each load is one
        # contiguous row per partition
        wr_d.append(nc.dram_tensor(f"wr{li}", [128, kt * dout], f16,
                                   kind="ExternalInput").ap())
        wl_d.append(nc.dram_tensor(f"wl{li}", [128, kt * dout], f16,
                                   kind="ExternalInput").ap())
        b_d.append(nc.dram_tensor(f"b{li}", [128, pt], f32,
                                  kind="ExternalInput").ap())
    out_d = nc.dram_tensor("out", [DOUT, RPC], f32, kind="ExternalOutput").ap()
    out_r = out_d.rearrange("(pt p) c -> p pt c", p=128)

    with tile.TileContext(nc) as tc:
        with (
            tc.tile_pool(name="consts", bufs=1) as consts,
            tc.tile_pool(name="xk", bufs=1) as xkp,
            tc.tile_pool(name="xs", bufs=6) as xsp,
            tc.tile_pool(name="pa", bufs=1) as pap,
            tc.tile_pool(name="ab", bufs=13) as abp,
            tc.tile_pool(name="raw", bufs=8) as rawp,
            tc.tile_pool(name="gsb", bufs=8) as gsbp,
            tc.tile_pool(name="sq", bufs=8) as sqp,
            tc.tile_pool(name="sbc", bufs=8) as sbcp,
            tc.tile_pool(name="ost", bufs=3) as ostp,
            tc.tile_pool(name="mainp", bufs=4, space="PSUM") as mainp,
            tc.tile_pool(name="ssp", bufs=2, space="PSUM") as sspp,
            tc.tile_pool(name="gp", bufs=2, space="PSUM") as gpp,
        ):
            # ---- load order matters: the first PE work is layer-1 G
            # (needs wl0 + xk) then mains (wr0, then xs chunks) ----
            wr_sb = [None] * 4
            wl_sb = [None] * 4
            b_sb = [None] * 4

            def load_w(lst, dram, li, kt, dout, nm):
                w = consts.tile([128, kt, dout], f16, tag=f"{nm}{li}",
                                name=f"{nm}{li}")
                nc.sync.dma_start(
                    out=w, in_=dram[li].rearrange("p (k d) -> p k d", k=kt))
                lst[li] = w

            def load_b(li, dout):
                bt = consts.tile([128, dout // 128], f32, tag=f"b{li}",
                                 name=f"b{li}")
                nc.sync.dma_start(out=bt, in_=b_d[li])
                b_sb[li] = bt

            xs_sb = {}

            def load_xs(ch):
                x1 = xsp.tile([128, P // 128, CH], f16, tag="xs",
                              name=f"xs{ch}")
                nc.sync.dma_start(
                    out=x1,
                    in_=xt.rearrange("(k p) c -> p k c",
                                     p=128)[:, :, ch * CH:(ch + 1) * CH])
                xs_sb[ch] = x1

            # coupled input first (feeds the first G matmuls), then
            # weights, M blocks, and the streamed plain chunks
            xk_sb = xkp.tile([128, P // 128, K], f16, tag="xk")
            nc.sync.dma_start(
                out=xk_sb,
                in_=xt.rearrange("(k p) c -> p k c", p=128)[:, :, 0:K])
            for li, (din, dout) in enumerate(DIMS):
                kt = din // 128
                load_w(wr_sb, wr_d, li, kt, dout, "wr")
                load_w(wl_sb, wl_d, li, kt, dout, "wl")
                load_b(li, dout)
            mt_sb = consts.tile([128, len(MT_BLOCKS), CH], f16, tag="mt")
            nc.sync.dma_start(
                out=mt_sb,
                in_=mt.rearrange("p (b c) -> p b c", b=len(MT_BLOCKS)))
            # all-ones stationary: the sumsq matmul then sums over features
            # AND broadcasts the result to every partition in one op
            ones128 = consts.tile([128, 128], f16, tag="ones128")
            nc.vector.memset(ones128, 1.0)
            for ch in range(2, NCH):
                load_xs(ch)

            # persistent coupled activations per layer (columns 0..1023)
            pa_sb = []
            for li, (din, dout) in enumerate(DIMS[:-1]):
                pa_sb.append(pap.tile([128, dout // 128, K], f16,
                                      tag=f"pa{li}", name=f"pa{li}"))

            # per-chunk current activation APs: [128, kt, CH] views
            ain = {}
            for ch in range(NCH):
                if ch < 2:
                    ain[ch] = xk_sb[:, :, ch * CH:(ch + 1) * CH]
                else:
                    ain[ch] = xs_sb[ch]

            for li, (din, dout) in enumerate(DIMS):
                ktn = din // 128
                ptn = dout // 128
                is_last = li == 3
                gain = xk_sb if li == 0 else pa_sb[li - 1]

                # --- coupled G: G[i, f] = x_k @ w_l.T, row(i)-major ---
                g_sb = []

                def emit_g():
                    for it in range(8):
                        gp = gpp.tile([128, dout], f32, tag="gp", name="gp")
                        for kt in range(ktn):
                            nc.tensor.matmul(
                                gp, lhsT=gain[:, kt, it * 128:(it + 1) * 128],
                                rhs=wl_sb[li][:, kt, :],
                                start=(kt == 0), stop=(kt == ktn - 1))
                        g = gsbp.tile([128, dout], f16, tag="g", name="g")
                        nc.scalar.copy(g, gp)
                        g_sb.append(g)

                order = list(range(NCH))
                emit_g()

                # --- S0: mains (+ C' for coupled chunks) ---
                mains = {}
                for ch in order:
                    for pt in range(ptn):
                        mp = mainp.tile([128, CH], f32, tag="mp",
                                        name=f"mp{ch}_{pt}")
                        for kt in range(ktn):
                            nc.tensor.matmul(
                                mp,
                                lhsT=wr_sb[li][:, kt, pt * 128:(pt + 1) * 128],
                                rhs=ain[ch][:, kt, :],
                                start=(kt == 0),
                                stop=(kt == ktn - 1 and ch >= 2))
                        if ch < 2:
                            its = [it for (it, j) in MT_BLOCKS if j == ch]
                            for ii, it in enumerate(its):
                                nc.tensor.matmul(
                                    mp,
                                    lhsT=g_sb[it][:, pt * 128:(pt + 1) * 128],
                                    rhs=mt_sb[:, _mt_block_id(it, ch), :],
                                    start=False, stop=(ii == len(its) - 1))
                        mains[(ch, pt)] = mp

                    # S1 immediately per chunk: raw16 = main + b (frees PSUM).
                    # Alternate ACT/DVE to balance engine load.
                    raw = rawp.tile([128, ptn, CH], f16, tag="raw",
                                    name=f"raw{ch}")
                    for pt in range(ptn):
                        if pt % 2 == 0:
                            nc.scalar.activation(
                                out=raw[:, pt, :], in_=mains[(ch, pt)],
                                func=AF.Identity,
                                bias=b_sb[li][:, pt:pt + 1], scale=1.0)
                        else:
                            nc.vector.tensor_scalar_add(
                                out=raw[:, pt, :], in0=mains[(ch, pt)],
                                scalar1=b_sb[li][:, pt:pt + 1])
                    mains[ch] = raw

                # --- S2: sq = (raw+b)^2 on DVE; S3: ss += ones.T @ sq ---
                # The sumsq matmuls trail the sq ops by two chunks so the
                # PE keeps streaming while early chunks' norm chains drain;
                # by the time the next layer's G matmuls need chunk 0/1's
                # outputs they are already done.
                sss = {}
                sqs = {}

                def emit_ss(ch):
                    ss = sspp.tile([128, CH], f32, tag="ss", name=f"ss{ch}")
                    for pt in range(ptn):
                        nc.tensor.matmul(ss, lhsT=ones128,
                                         rhs=sqs[ch][:, pt, :],
                                         start=(pt == 0), stop=(pt == ptn - 1))
                    sss[ch] = ss

                for ch in order:
                    raw = mains[ch]
                    sq = sqp.tile([128, ptn, CH], f16, tag="sq",
                                  name=f"sq{ch}")
                    for pt in range(ptn):
                        nc.vector.tensor_mul(
                            out=sq[:, pt, :], in0=raw[:, pt, :],
                            in1=raw[:, pt, :])
                    sqs[ch] = sq
                for ch in order:
                    emit_ss(ch)

                # --- S4: rsqrt of the broadcast sumsq, one wide ACT op ---
                sbs = {}
                for ch in order:
                    sb = sbcp.tile([128, CH], f16, tag="sbc", name=f"sb{ch}")
                    nc.scalar.activation(out=sb, in_=sss[ch],
                                         func=AF.Abs_reciprocal_sqrt)
                    sbs[ch] = sb

                # --- S7: apply scale ---
                for ch in order:
                    raw = mains[ch]
                    sb = sbs[ch]
                    if not is_last:
                        if ch < 2:
                            aout = pa_sb[li]
                            asl = (slice(None), slice(None),
                                   slice(ch * CH, (ch + 1) * CH))
                        else:
                            anext = abp.tile([128, ptn, CH], f16, tag="ab",
                                             name=f"ab{ch}")
                            aout = anext
                            asl = (slice(None), slice(None), slice(0, CH))
                            ain[ch] = anext
                        for pt in range(ptn):
                            dst = aout[asl[0], pt, asl[2]]
                            nc.vector.tensor_mul(
                                out=dst, in0=raw[:, pt, :], in1=sb)
                        if ch < 2:
                            ain[ch] = pa_sb[li][:, :,
                                               ch * CH:(ch + 1) * CH]
                    else:
                        ost = ostp.tile([128, ptn, CH], f32, tag="ost",
                                        name=f"ost{ch}")
                        for pt in range(ptn):
                            # relu((raw+b)*s) = max(raw+b,0)*s since s>0
                            nc.vector.scalar_tensor_tensor(
                                out=ost[:, pt, :], in0=raw[:, pt, :],
                                scalar=0.0, in1=sb, op0=OP.max, op1=OP.mult)
                        nc.sync.dma_start(
                            out=out_r[:, :, ch * CH:(ch + 1) * CH], in_=ost)

    nc.compile()
    return nc


def _prep_inputs(flow_matrix, ws):
    """ws: list of (w_l, b_l, w_r) fp32. Returns list of 8 in_maps."""
    x = np.ascontiguousarray(flow_matrix.reshape(B * K, P))
    xt_full = np.ascontiguousarray(x.T.astype(np.float16))  # [P, 32768]

    # M.T packed nonzero blocks, fp16
    inv = np.zeros(K, np.float32)
    inv[1:] = 1.0 / np.arange(1, K, dtype=np.float32)
    mt_packed = np.zeros((128, len(MT_BLOCKS) * CH), np.float16)
    for bid, (it, jc) in enumerate(MT_BLOCKS):
        i0, j0 = it * 128, jc * CH
        blk = np.zeros((128, CH), np.float32)
        for pp in range(128):
            i = i0 + pp
            jj = np.arange(j0, j0 + CH)
            blk[pp] = np.where(jj > i, inv[jj], 0.0)
        mt_packed[:, bid * CH:(bid + 1) * CH] = blk.astype(np.float16)
    mt_zero = np.zeros_like(mt_packed)

    def pack_w(wt):  # [din, dout] -> [128, kt*dout] partition-major
        din, dout = wt.shape
        kt = din // 128
        return np.ascontiguousarray(
            wt.reshape(kt, 128, dout).transpose(1, 0, 2).reshape(128, -1)
            .astype(np.float16))

    base = {}
    for li, (w_l, b_l, w_r) in enumerate(ws):
        base[f"wr{li}"] = pack_w(w_r.T)
        base[f"wl{li}"] = pack_w(w_l.T)
        base[f"b{li}"] = np.ascontiguousarray(
            b_l.reshape(-1, 128).T.astype(np.float32))

    in_maps = []
    for c in range(N_CORES):
        m = dict(base)
        m["xt"] = np.ascontiguousarray(xt_full[:, c * RPC:(c + 1) * RPC])
        m["mt"] = mt_packed if c == 0 else mt_zero
        in_maps.append(m)
    return in_maps


def kernel(flow_matrix, w_l1, b_l1, w_r1, w_l2, b_l2, w_r2,
           w_l3, b_l3, w_r3, w_l4, b_l4, w_r4, _trace=False, _tmpdir=None):
    from concourse import bass_utils

    flow_matrix = np.asarray(flow_matrix, dtype=np.float32)
    ws = [(np.asarray(w_l1, np.float32), np.asarray(b_l1, np.float32),
           np.asarray(w_r1, np.float32)),
          (np.asarray(w_l2, np.float32), np.asarray(b_l2, np.float32),
           np.asarray(w_r2, np.float32)),
          (np.asarray(w_l3, np.float32), np.asarray(b_l3, np.float32),
           np.asarray(w_r3, np.float32)),
          (np.asarray(w_l4, np.float32), np.asarray(b_l4, np.float32),
           np.asarray(w_r4, np.float32))]

    if "nc" not in _CACHE:
        _CACHE["nc"] = _build_program()
    nc = _CACHE["nc"]

    in_maps = _prep_inputs(flow_matrix, ws)
    res = None
    for attempt in range(3):
        try:
            res = bass_utils.run_bass_kernel_spmd(
                nc, in_maps, core_ids=list(range(N_CORES)), trace=_trace,
                tmpdir=_tmpdir)
            break
        except Exception:
            # the axon-tunneled device occasionally reports a transient
            # NRT_EXEC_UNIT_UNRECOVERABLE; a fresh dispatch succeeds
            if attempt == 2:
                raise
            import time
            time.sleep(2.0)

    y = np.empty((B * K, DOUT), np.float32)
    for c in range(N_CORES):
        out_c = res.results[c]["out"]  # [DOUT, RPC] fp32
        y[c * RPC:(c + 1) * RPC, :] = out_c.T
    _CACHE["last_exec_time_ns"] = res.exec_time_ns
    return (y.reshape(B, K, DOUT), 1)


if __name__ == "__main__":
    rng = np.random.default_rng(0)
    fm = rng.standard_normal((B, K, P)).astype(np.float32)
    args = []
    for (din, dout) in DIMS:
        s = 1.0 / np.sqrt(din)
        args += [rng.uniform(-s, s, (dout, din)).astype(np.float32),
                 rng.uniform(-s, s, dout).astype(np.float32),
                 rng.uniform(-s, s, (dout, din)).astype(np.float32)]
    y, _ = kernel(fm, *args)
    print("ok", y.shape, y.dtype)


# revision 39
# speedup vs baseline: 1.2756x; 1.0248x over previous
"""Bass/Trainium2 kernel for nn_Encoder_Flows (4-layer SAGE encoder with
buggy prefix-mean aggregation), SPMD over 8 NeuronCores.

Math (per reference):
  x = flow_matrix.reshape(B*K, P)   # [32768, 1024]
  4x: out = agg @ w_l.T + b_l + x @ w_r.T ; out /= ||out||_row
  where agg[j] = mean_{i<j} x[i] for j < K=1024, else 0.
  final relu.

Strategy:
  - Shard the 32768 rows into 8 contiguous shards of 4096 (core c gets rows
    [4096c, 4096(c+1))). Rows >= 1024 are row-independent (agg = 0).
  - Feature-major on chip: activations live as A[d, cols]; matmuls are then
    always out[dout_tile, cols] = W_T_slice.T @ A with no transposes.
  - The prefix-mean for rows < 1024 (core 0 only) is a matmul against a
    lower-triangular coefficient matrix M (L[j,i] = 1/j, i<j):
      C = G contracted against M.T, with G = x_k @ w_l.T computed row-major
      (its lhsT is exactly the feature-major activation tile).
    Cores 1-7 get M = 0 (same SPMD code, zero contribution).
  - fp16 matmul operands (full PE rate), fp32 PSUM accumulate, fp32 output.
  - All 8 column-chunks march through the layers together, stage by stage
    (the Tile scheduler follows emission order per engine, so emission
    interleaving IS the software pipeline).
  - Row norm (per free-dim column): bias+copy to fp16 (frees PSUM fast),
    square on DVE, column-sum via ones-vector matmul on PE, sqrt on ACT,
    reciprocal on DVE on a [128, CH/128] refold (a [1, CH] strip would be
    single-lane), partition-broadcast on GpSimd, fused multiply on DVE.
"""

import sys

if "/opt/trn_rl_repo" not in sys.path:
    sys.path.insert(0, "/opt/trn_rl_repo")

import numpy as np

B, K, P = 32, 1024, 1024
N_CORES = 8
RPC = (B * K) // N_CORES  # 4096 columns (rows of x) per core
CH = 512                  # chunk of columns processed at once
NCH = RPC // CH           # 8 chunks; chunks 0,1 hold the coupled rows 0..1023
DIMS = [(1024, 128), (128, 256), (256, 128), (128, 256)]
DOUT = DIMS[-1][1]

# nonzero [128i, 512j] blocks of M.T (MT[i,j] = 1/j if i<j else 0)
MT_BLOCKS = [(it, 0) for it in range(4)] + [(it, 1) for it in range(8)]


def _mt_block_id(it, jc):
    return it if jc == 0 else 4 + it


_CACHE = {}


def _build_program():
    import concourse.bass as bass  # noqa: F401
    import concourse.tile as tile
    from concourse import bacc, mybir

    f16 = mybir.dt.float16
    f32 = mybir.dt.float32
    AF = mybir.ActivationFunctionType
    OP = mybir.AluOpType

    nc = bacc.Bacc("TRN2", target_bir_lowering=False, debug=False)

    xt = nc.dram_tensor("xt", [P, RPC], f16, kind="ExternalInput").ap()
    mt = nc.dram_tensor("mt", [128, len(MT_BLOCKS) * CH], f16,
                        kind="ExternalInput").ap()
    wr_d, wl_d, b_d = [], [], []
    for li, (din, dout) in enumerate(DIMS):
        kt, pt = din // 128, dout // 128
        # host pre-packs weights partition-major so each load is one
        # contiguous row per partition
        wr_d.append(nc.dram_tensor(f"wr{li}", [128, kt * dout], f16,
                                   kind="ExternalInput").ap())
        wl_d.append(nc.dram_tensor(f"wl{li}", [128, kt * dout], f16,
                                   kind="ExternalInput").ap())
        b_d.append(nc.dram_tensor(f"b{li}", [128, pt], f32,
                                  kind="ExternalInput").ap())
    out_d = nc.dram_tensor("out", [DOUT, RPC], f32, kind="ExternalOutput").ap()
    out_r = out_d.rearrange("(pt p) c -> p pt c", p=128)

    with tile.TileContext(nc) as tc:
        with (
            tc.tile_pool(name="consts", bufs=1) as consts,
            tc.tile_pool(name="xk", bufs=1) as xkp,
            tc.tile_pool(name="xs", bufs=6) as xsp,
            tc.tile_pool(name="pa", bufs=1) as pap,
            tc.tile_pool(name="ab", bufs=13) as abp,
            tc.tile_pool(name="raw", bufs=8) as rawp,
            tc.tile_pool(name="gsb", bufs=8) as gsbp,
            tc.tile_pool(name="sq", bufs=8) as sqp,
            tc.tile_pool(name="sbc", bufs=8) as sbcp,
            tc.tile_pool(name="ost", bufs=3) as ostp,
            tc.tile_pool(name="mainp", bufs=4, space="PSUM") as mainp,
            tc.tile_pool(name="ssp", bufs=2, space="PSUM") as sspp,
            tc.tile_pool(name="gp", bufs=2, space="PSUM") as gpp,
        ):
            # ---- load order matters: the first PE work is layer-1 G
            # (needs wl0 + xk) then mains (wr0, then xs chunks) ----
            wr_sb = [None] * 4
            wl_sb = [None] * 4
            b_sb = [None] * 4

            def load_w(lst, dram, li, kt, dout, nm):
                w = consts.tile([128, kt, dout], f16, tag=f"{nm}{li}",
                                name=f"{nm}{li}")
                nc.sync.dma_start(
                    out=w, in_=dram[li].rearrange("p (k d) -> p k d", k=kt))
                lst[li] = w

            def load_b(li, dout):
                bt = consts.tile([128, dout // 128], f32, tag=f"b{li}",
                                 name=f"b{li}")
                nc.sync.dma_start(out=bt, in_=b_d[li])
                b_sb[li] = bt

            xs_sb = {}

            def load_xs(ch):
                x1 = xsp.tile([128, P // 128, CH], f16, tag="xs",
                              name=f"xs{ch}")
                nc.sync.dma_start(
                    out=x1,
                    in_=xt.rearrange("(k p) c -> p k c",
                                     p=128)[:, :, ch * CH:(ch + 1) * CH])
                xs_sb[ch] = x1

            # coupled input first (feeds the first G matmuls): wl0 (its
            # weights), then x.T cols 0..1023 in two halves so the first
            # four G i-tiles can start after half one
            load_w(wl_sb, wl_d, 0, 8, 128, "wl")
            xk_sb = xkp.tile([128, P // 128, K], f16, tag="xk")
            xk_r = xt.rearrange("(k p) c -> p k c", p=128)
            nc.sync.dma_start(out=xk_sb[:, :, 0:K // 2],
                              in_=xk_r[:, :, 0:K // 2])
            nc.sync.dma_start(out=xk_sb[:, :, K // 2:K],
                              in_=xk_r[:, :, K // 2:K])
            for li, (din, dout) in enumerate(DIMS):
                kt = din // 128
                load_w(wr_sb, wr_d, li, kt, dout, "wr")
                if li > 0:
                    load_w(wl_sb, wl_d, li, kt, dout, "wl")
                load_b(li, dout)
            mt_sb = consts.tile([128, len(MT_BLOCKS), CH], f16, tag="mt")
            nc.sync.dma_start(
                out=mt_sb,
                in_=mt.rearrange("p (b c) -> p b c", b=len(MT_BLOCKS)))
            # all-ones stationary: the sumsq matmul then sums over features
            # AND broadcasts the result to every partition in one op
            ones128 = consts.tile([128, 128], f16, tag="ones128")
            nc.vector.memset(ones128, 1.0)
            for ch in range(2, NCH):
                load_xs(ch)

            # persistent coupled activations per layer (columns 0..1023)
            pa_sb = []
            for li, (din, dout) in enumerate(DIMS[:-1]):
                pa_sb.append(pap.tile([128, dout // 128, K], f16,
                                      tag=f"pa{li}", name=f"pa{li}"))

            # per-chunk current activation APs: [128, kt, CH] views
            ain = {}
            for ch in range(NCH):
                if ch < 2:
                    ain[ch] = xk_sb[:, :, ch * CH:(ch + 1) * CH]
                else:
                    ain[ch] = xs_sb[ch]

            for li, (din, dout) in enumerate(DIMS):
                ktn = din // 128
                ptn = dout // 128
                is_last = li == 3
                gain = xk_sb if li == 0 else pa_sb[li - 1]

                # --- coupled G: G[i, f] = x_k @ w_l.T, row(i)-major ---
                g_sb = []

                def emit_g():
                    for it in range(8):
                        gp = gpp.tile([128, dout], f32, tag="gp", name="gp")
                        for kt in range(ktn):
                            nc.tensor.matmul(
                                gp, lhsT=gain[:, kt, it * 128:(it + 1) * 128],
                                rhs=wl_sb[li][:, kt, :],
                                start=(kt == 0), stop=(kt == ktn - 1))
                        g = gsbp.tile([128, dout], f16, tag="g", name="g")
                        nc.scalar.copy(g, gp)
                        g_sb.append(g)

                order = list(range(NCH))
                emit_g()

                # --- S0: mains (+ C' for coupled chunks) ---
                mains = {}
                for ch in order:
                    for pt in range(ptn):
                        mp = mainp.tile([128, CH], f32, tag="mp",
                                        name=f"mp{ch}_{pt}")
                        for kt in range(ktn):
                            nc.tensor.matmul(
                                mp,
                                lhsT=wr_sb[li][:, kt, pt * 128:(pt + 1) * 128],
                                rhs=ain[ch][:, kt, :],
                                start=(kt == 0),
                                stop=(kt == ktn - 1 and ch >= 2))
                        if ch < 2:
                            its = [it for (it, j) in MT_BLOCKS if j == ch]
                            for ii, it in enumerate(its):
                                nc.tensor.matmul(
                                    mp,
                                    lhsT=g_sb[it][:, pt * 128:(pt + 1) * 128],
                                    rhs=mt_sb[:, _mt_block_id(it, ch), :],
                                    start=False, stop=(ii == len(its) - 1))
                        mains[(ch, pt)] = mp

                    # S1 immediately per chunk: raw16 = main + b (frees PSUM).
                    # Alternate ACT/DVE to balance engine load.
                    raw = rawp.tile([128, ptn, CH], f16, tag="raw",
                                    name=f"raw{ch}")
                    for pt in range(ptn):
                        if pt % 2 == 0:
                            nc.scalar.activation(
                                out=raw[:, pt, :], in_=mains[(ch, pt)],
                                func=AF.Identity,
                                bias=b_sb[li][:, pt:pt + 1], scale=1.0)
                        else:
                            nc.vector.tensor_scalar_add(
                                out=raw[:, pt, :], in0=mains[(ch, pt)],
                                scalar1=b_sb[li][:, pt:pt + 1])
                    mains[ch] = raw

                # --- S2: sq = (raw+b)^2 on DVE; S3: ss += ones.T @ sq ---
                # The sumsq matmuls trail the sq ops by two chunks so the
                # PE keeps streaming while early chunks' norm chains drain;
                # by the time the next layer's G matmuls need chunk 0/1's
                # outputs they are already done.
                sss = {}
                sqs = {}
                sbs = {}

                def emit_sq(ch):
                    raw = mains[ch]
                    sq = sqp.tile([128, ptn, CH], f16, tag="sq",
                                  name=f"sq{ch}")
                    for pt in range(ptn):
                        nc.vector.tensor_mul(
                            out=sq[:, pt, :], in0=raw[:, pt, :],
                            in1=raw[:, pt, :])
                    sqs[ch] = sq

                def emit_ss(ch):
                    ss = sspp.tile([128, CH], f32, tag="ss", name=f"ss{ch}")
                    for pt in range(ptn):
                        nc.tensor.matmul(ss, lhsT=ones128,
                                         rhs=sqs[ch][:, pt, :],
                                         start=(pt == 0), stop=(pt == ptn - 1))
                    sss[ch] = ss

                def emit_rsqrt(ch):
                    sb = sbcp.tile([128, CH], f16, tag="sbc", name=f"sb{ch}")
                    nc.scalar.activation(out=sb, in_=sss[ch],
                                         func=AF.Abs_reciprocal_sqrt)
                    sbs[ch] = sb

                def emit_apply(ch):
                    raw = mains[ch]
                    sb = sbs[ch]
                    if not is_last:
                        if ch < 2:
                            aout = pa_sb[li]
                            asl = slice(ch * CH, (ch + 1) * CH)
                        else:
                            anext = abp.tile([128, ptn, CH], f16, tag="ab",
                                             name=f"ab{ch}")
                            aout = anext
                            asl = slice(0, CH)
                            ain[ch] = anext
                        for pt in range(ptn):
                            nc.vector.tensor_mul(
                                out=aout[:, pt, asl], in0=raw[:, pt, :],
                                in1=sb)
                        if ch < 2:
                            ain[ch] = pa_sb[li][:, :,
                                               ch * CH:(ch + 1) * CH]
                    else:
                        ost = ostp.tile([128, ptn, CH], f32, tag="ost",
                                        name=f"ost{ch}")
                        for pt in range(ptn):
                            # relu((raw+b)*s) = max(raw+b,0)*s since s>0
                            nc.vector.scalar_tensor_tensor(
                                out=ost[:, pt, :], in0=raw[:, pt, :],
                                scalar=0.0, in1=sb, op0=OP.max, op1=OP.mult)
                        nc.sync.dma_start(
                            out=out_r[:, :, ch * CH:(ch + 1) * CH], in_=ost)

                # chunks 0/1 feed the next layer's G matmuls: push their
                # whole norm chain to the front of every engine queue so G
                # never stalls at the layer boundary
                for ch in (0, 1):
                    emit_sq(ch)
                    emit_ss(ch)
                    emit_rsqrt(ch)
                    emit_apply(ch)
                for ch in range(2, NCH):
                    emit_sq(ch)
                for ch in range(2, NCH):
                    emit_ss(ch)
                for ch in range(2, NCH):
                    emit_rsqrt(ch)
                for ch in range(2, NCH):
                    emit_apply(ch)

    nc.compile()
    return nc


def _prep_inputs(flow_matrix, ws):
    """ws: list of (w_l, b_l, w_r) fp32. Returns list of 8 in_maps."""
    x = np.ascontiguousarray(flow_matrix.reshape(B * K, P))
    xt_full = np.ascontiguousarray(x.T.astype(np.float16))  # [P, 32768]

    # M.T packed nonzero blocks, fp16
    inv = np.zeros(K, np.float32)
    inv[1:] = 1.0 / np.arange(1, K, dtype=np.float32)
    mt_packed = np.zeros((128, len(MT_BLOCKS) * CH), np.float16)
    for bid, (it, jc) in enumerate(MT_BLOCKS):
        i0, j0 = it * 128, jc * CH
        blk = np.zeros((128, CH), np.float32)
        for pp in range(128):
            i = i0 + pp
            jj = np.arange(j0, j0 + CH)
            blk[pp] = np.where(jj > i, inv[jj], 0.0)
        mt_packed[:, bid * CH:(bid + 1) * CH] = blk.astype(np.float16)
    mt_zero = np.zeros_like(mt_packed)

    def pack_w(wt):  # [din, dout] -> [128, kt*dout] partition-major
        din, dout = wt.shape
        kt = din // 128
        return np.ascontiguousarray(
            wt.reshape(kt, 128, dout).transpose(1, 0, 2).reshape(128, -1)
            .astype(np.float16))

    base = {}
    for li, (w_l, b_l, w_r) in enumerate(ws):
        base[f"wr{li}"] = pack_w(w_r.T)
        base[f"wl{li}"] = pack_w(w_l.T)
        base[f"b{li}"] = np.ascontiguousarray(
            b_l.reshape(-1, 128).T.astype(np.float32))

    in_maps = []
    for c in range(N_CORES):
        m = dict(base)
        m["xt"] = np.ascontiguousarray(xt_full[:, c * RPC:(c + 1) * RPC])
        m["mt"] = mt_packed if c == 0 else mt_zero
        in_maps.append(m)
    return in_maps


def kernel(flow_matrix, w_l1, b_l1, w_r1, w_l2, b_l2, w_r2,
           w_l3, b_l3, w_r3, w_l4, b_l4, w_r4, _trace=False, _tmpdir=None):
    from concourse import bass_utils

    flow_matrix = np.asarray(flow_matrix, dtype=np.float32)
    ws = [(np.asarray(w_l1, np.float32), np.asarray(b_l1, np.float32),
           np.asarray(w_r1, np.float32)),
          (np.asarray(w_l2, np.float32), np.asarray(b_l2, np.float32),
           np.asarray(w_r2, np.float32)),
          (np.asarray(w_l3, np.float32), np.asarray(b_l3, np.float32),
           np.asarray(w_r3, np.float32)),
          (np.asarray(w_l4, np.float32), np.asarray(b_l4, np.float32),
           np.asarray(w_r4, np.float32))]

    if "nc" not in _CACHE:
        _CACHE["nc"] = _build_program()
    nc = _CACHE["nc"]

    in_maps = _prep_inputs(flow_matrix, ws)
    res = None
    for attempt in range(3):
        try:
            res = bass_utils.run_bass_kernel_spmd(
                nc, in_maps, core_ids=list(range(N_CORES)), trace=_trace,
                tmpdir=_tmpdir)
            break
        except Exception:
            # the axon-tunneled device occasionally reports a transient
            # NRT_EXEC_UNIT_UNRECOVERABLE; a fresh dispatch succeeds
            if attempt == 2:
                raise
            import time
            time.sleep(2.0)

    y = np.empty((B * K, DOUT), np.float32)
    for c in range(N_CORES):
        out_c = res.results[c]["out"]  # [DOUT, RPC] fp32
        y[c * RPC:(c + 1) * RPC, :] = out_c.T
    _CACHE["last_exec_time_ns"] = res.exec_time_ns
    return (y.reshape(B, K, DOUT), 1)


if __name__ == "__main__":
    rng = np.random.default_rng(0)
    fm = rng.standard_normal((B, K, P)).astype(np.float32)
    args = []
    for (din, dout) in DIMS:
        s = 1.0 / np.sqrt(din)
        args += [rng.uniform(-s, s, (dout, din)).astype(np.float32),
                 rng.uniform(-s, s, dout).astype(np.float32),
                 rng.uniform(-s, s, (dout, din)).astype(np.float32)]
    y, _ = kernel(fm, *args)
    print("ok", y.shape, y.dtype)
